# revision 14
# baseline (speedup 1.0000x reference)
"""Trainium2 Bass kernel for nn_ChargesReadoutBlock.

Math: the reference is
    y_l = (x_l @ W_lin_l) / sqrt(256)            (per irrep l = 0e, 1o, 2e)
    p_l = einsum('zui,u,zi->z', y_l, W_tp_l, c_l) / sqrt(2l+1)
    energy = (p_0 + p_1 + p_2) / sqrt(3*256)

Everything is linear, so the two weight stages collapse on the host:
    w_l = W_lin_l @ W_tp_l                       ([256] each)
    energy[z] = sum_k node_feats[z, k] * A[k] * C[z, j(k)]
where A folds w_l and all scalar norms (per-k weight, shared across nodes)
and C[z, j] are the 9 per-node charge components (j = irrep slot of k).

Device kernel (data-parallel over nodes, 8 cores x 8192 nodes): nodes live
on partitions (128) x 64 node-tiles. The 9 irrep-component sections are
split across two engines so each stays under the HBM DMA roofline:

  ACT (6 sections: all 5 of l2, plus l1 i=2): per section
      y[:, s*256:(s+1)*256] = x_section * C[z, j]     (activation, scale=AP)
  DVE (1 batched reduce + 3 direct sections):
      big STT:  accum  = sum(y * awt_grouped)          FD=1536, one op/tile
      3x STT:   accum_j = sum((x_sec * C[z,j]) * w_sec) FD=256  (l0, l1 i=0,1)
      per chunk: tensor_reduce over the 4 accum slots -> energy

awt layout (host-folded): [w0n | w1n | w1n | w2n x5 | w1n], so the big STT's
in1 is contiguous and matches y's section order [l2_i0..4, l1_i2].
All heavy traffic is the single streaming read of node_feats
(contiguous 73.7KB/partition DMA chunks).
"""

import numpy as np

N_NODES = 65536
N_CORES = 8
MUL = 256
K = 9 * MUL            # 2304 features per node
P = 128                # SBUF partitions
N_SHARD = N_NODES // N_CORES   # 8192 nodes per core
T = N_SHARD // P       # 64 node-tiles per partition (node = p*T + t)
G = 4                  # node-tiles per DMA chunk
SQ3 = float(np.sqrt(3.0))
SQ5 = float(np.sqrt(5.0))

_PROGRAM_CACHE = {}
LAST_RESULTS = None    # BassKernelResults of the most recent kernel() call


def build_program(t_tiles=T, g_chunk=G, repeat=1, hw_loop=1,
                  skip_compute=False, dma_once=False):
    """Build the SPMD Bass program (same program for every core).

    repeat > 1 unrolls the whole body `repeat` times (straight-line, same
    buffers, identical output) — used only by the timing harness to
    amortize host dispatch overhead out of the measurement. hw_loop > 1
    additionally wraps the unrolled body in a device-side For_i loop
    (cheap to compile; pays one all-engine barrier per back edge, which
    `repeat` amortizes).
    """
    import concourse.bass as bass
    import concourse.tile as tile
    from concourse import mybir

    f32 = mybir.dt.float32
    mult = mybir.AluOpType.mult
    copy_fn = mybir.ActivationFunctionType.Copy

    nc = bass.Bass(trn_type="TRN2", debug=False, name="charges_readout")
    x = nc.dram_tensor("x", [P, t_tiles * K], f32, kind="ExternalInput").ap()
    arep = nc.dram_tensor("arep", [P, K], f32, kind="ExternalInput").ap()
    ch = nc.dram_tensor("ch", [P, t_tiles * 9], f32, kind="ExternalInput").ap()
    en = nc.dram_tensor("en", [P, t_tiles], f32, kind="ExternalOutput").ap()

    n_chunks = t_tiles // g_chunk

    with tile.TileContext(nc) as tc:
        with tc.tile_pool(name="const", bufs=1) as cpool, \
             tc.tile_pool(name="xp", bufs=2) as xpool, \
             tc.tile_pool(name="yp", bufs=2) as ypool, \
             tc.tile_pool(name="scrp", bufs=2) as spool, \
             tc.tile_pool(name="s4p", bufs=2) as s4pool:

            awt_t = cpool.tile([P, K], f32)
            nc.sync.dma_start(out=awt_t[:], in_=arep[:, :])
            ch_t = cpool.tile([P, t_tiles * 9], f32)
            nc.sync.dma_start(out=ch_t[:], in_=ch[:, :])
            en_t = cpool.tile([P, t_tiles], f32)

            # Wait-collectors: absorb the const-DMA completion waits on cheap
            # copy ops so the first real compute op on each engine doesn't
            # accumulate more sync-wait slots than its ISA struct allows.
            dummy = cpool.tile([P, 3], f32)
            nc.vector.tensor_copy(dummy[:, 0:1], awt_t[:, 0:1])
            nc.vector.tensor_copy(dummy[:, 1:2], ch_t[:, 0:1])
            nc.scalar.copy(dummy[:, 2:3], ch_t[:, 1:2])
            if skip_compute:
                nc.vector.memset(en_t[:], 0.0)

            xg_cache = []

            def emit_rep():
                for c in range(n_chunks):
                    # dma_once: load only the first two chunks, then cycle
                    # those buffers for all compute (compute-only timing probe)
                    if dma_once and len(xg_cache) >= 2:
                        xg = xg_cache[c % 2]
                    else:
                        xg = xpool.tile([P, g_chunk * K], f32)
                        nc.sync.dma_start(
                            out=xg[:],
                            in_=x[:, c * g_chunk * K:(c + 1) * g_chunk * K]
                        )
                        if dma_once:
                            xg_cache.append(xg)
                    if skip_compute:
                        # tiny DVE read per chunk keeps the reader-release
                        # wait chain intact (prunable single-wait DMAs)
                        sink = spool.tile([P, 1], f32)
                        nc.vector.tensor_copy(sink[:], xg[:, 0:1])
                        continue
                    s4c = s4pool.tile([P, g_chunk * 4], f32)
                    yb = ypool.tile([P, g_chunk * 6 * MUL], f32)
                    ylast = g_chunk * 6 * MUL
                    # ACT wait-collectors (ISA allows 1 wait per ACT op):
                    # first absorbs the ybig WAR release (vs the big STTs of
                    # the chunk that last used this buffer) at its maximum
                    # value; second absorbs this chunk's x-DMA completion.
                    # Both write junk into the last tile's last section,
                    # which the real ops fully overwrite before it is read.
                    nc.scalar.activation(
                        out=yb[:, ylast - 1:ylast], in_=ch_t[:, 0:1],
                        func=copy_fn)
                    nc.scalar.activation(
                        out=yb[:, ylast - 2:ylast - 1], in_=xg[:, 0:1],
                        func=copy_fn)
                    for tsub in range(g_chunk):
                        t_idx = c * g_chunk + tsub
                        t9 = t_idx * 9
                        base = tsub * K
                        y = yb[:, tsub * 6 * MUL:(tsub + 1) * 6 * MUL]
                        scr = spool.tile([P, K], f32)
                        x1 = xg[:, base + MUL:base + 4 * MUL].rearrange(
                            "p (u i) -> p i u", i=3)
                        x2 = xg[:, base + 4 * MUL:base + 9 * MUL].rearrange(
                            "p (u i) -> p i u", i=5)
                        # ACT: charge-scale the 5 l2 sections + l1 i=2 into y
                        for s in range(5):
                            nc.scalar.activation(
                                out=y[:, s * MUL:(s + 1) * MUL],
                                in_=x2[:, s, :],
                                func=copy_fn,
                                scale=ch_t[:, t9 + 4 + s:t9 + 5 + s],
                            )
                        nc.scalar.activation(
                            out=y[:, 5 * MUL:6 * MUL],
                            in_=x1[:, 2, :],
                            func=copy_fn,
                            scale=ch_t[:, t9 + 3:t9 + 4],
                        )
                        # DVE: 3 direct fused sections (l0, l1 i=0, l1 i=1)
                        direct = [
                            (xg[:, base:base + MUL], 0, 0),
                            (x1[:, 0, :], 1, 1),
                            (x1[:, 1, :], 2, 2),
                        ]
                        for xi, ci, j in direct:
                            nc.vector.scalar_tensor_tensor(
                                out=scr[:, j * MUL:(j + 1) * MUL],
                                in0=xi,
                                scalar=ch_t[:, t9 + ci:t9 + ci + 1],
                                in1=awt_t[:, j * MUL:(j + 1) * MUL],
                                op0=mult,
                                op1=mult,
                                accum_out=s4c[:, tsub * 4 + 1 + j:tsub * 4 + 2 + j],
                            )
                        # DVE: one batched reduce over ACT's 6 sections
                        nc.vector.scalar_tensor_tensor(
                            out=scr[:, 3 * MUL:9 * MUL],
                            in0=y[:],
                            scalar=1.0,
                            in1=awt_t[:, 3 * MUL:9 * MUL],
                            op0=mult,
                            op1=mult,
                            accum_out=s4c[:, tsub * 4:tsub * 4 + 1],
                        )
                    # one batched reduce per chunk: [P, g, 4] -> [P, g]
                    nc.vector.tensor_reduce(
                        out=en_t[:, c * g_chunk:(c + 1) * g_chunk],
                        in_=s4c[:].rearrange("p (g j) -> p g j", j=4),
                        axis=mybir.AxisListType.X,
                        op=mybir.AluOpType.add,
                    )

            if hw_loop > 1:
                with tc.For_i(0, hw_loop, 1):
                    for _rep in range(repeat):
                        emit_rep()
            else:
                for _rep in range(repeat):
                    emit_rep()
            nc.sync.dma_start(out=en[:, :], in_=en_t[:])
    _prune_implied_dma_waits(nc)
    return nc


def _prune_implied_dma_waits(nc):
    """Reduce sync waits to what the ISA wait slots allow.

    Walrus in this toolchain rejects DMAs with more than one sync wait and
    DVE/ACT compute ops with more than two. Tile's wait emission is per-proc
    minimal but not transitively minimal (documented), so we drop waits we
    can prove implied:
      (a) own-proc-sem waits with value <= updates already completed by the
          same in-order engine (same-engine WAW/WAR ordering);
      (b) waits an earlier same-engine instruction already carried (issue
          order is program order, waits are satisfied before issue);
      (c) waits implied transitively through another carried wait (the
          instruction that pushed sem X to the waited value itself carried,
          or transitively implies, the other wait);
      (d) two-reader DMA releases: anchor on one proc-sem wait, raising it
          minimally so that engine's in-order stream implies the other.
    Each basic block is processed as an independent region: For_i loop
    bodies repeat with per-iteration semaphore resets, so cross-block state
    must not leak. Tile's own loop machinery (NoOp/Drain barriers in
    reset/exit blocks) is left untouched.
    """
    import bisect

    from concourse import mybir

    # The program is straight-line (no For_i): treat all blocks as one
    # linear region. (Looped programs would need per-iteration state
    # resets; the For_i path is unused — walrus rejects its barrier ops.)
    class _Lin:
        instructions = [i for b in nc.m.functions[0].blocks
                        for i in b.instructions]

    _prune_block(_Lin, mybir, bisect)
    return nc


def _prune_block(blk, mybir, bisect):
    compute_engines = (mybir.EngineType.DVE, mybir.EngineType.Activation)

    # Pass 1: per-engine instruction streams in program order with
    # cumulative proc-sem ticks and the waits each instruction carried.
    eng_sems = {}
    eng_streams = {}
    eng_cum = {}
    for inst in blk.instructions:
        eng = inst.engine
        if eng not in compute_engines:
            continue
        si = inst.sync_info
        waits = [(w.ant_name, w.wait_value) for w in (si.on_wait or [])] \
            if si else []
        if si and si.on_update:
            for u in si.on_update:
                if u.ant_name.startswith(("DVE", "ACT", "Activation")):
                    eng_sems[eng] = u.ant_name
                    eng_cum[eng] = eng_cum.get(eng, 0) + u.update_value
        eng_streams.setdefault(eng, []).append((eng_cum.get(eng, 0), waits))

    # Index each engine stream for O(log n) implication queries.
    sem_to_index = {}
    for eng, sem in eng_sems.items():
        stream = eng_streams[eng]
        cums = [cum for cum, _ in stream]
        per_sem = {}
        for idx, (_, waits) in enumerate(stream):
            for s, v in waits:
                poss, pmax = per_sem.setdefault(s, ([], []))
                poss.append(idx)
                pmax.append(max(v, pmax[-1]) if pmax else v)
        sem_to_index[sem] = (cums, per_sem)

    # Per DMA-lane sem: (cumulative completion value, carried waits).
    dma_lane = {}
    for inst in blk.instructions:
        if inst.opcode != "DMACopy":
            continue
        si = inst.sync_info
        if not si or not si.on_update:
            continue
        waits = [(w.ant_name, w.wait_value) for w in (si.on_wait or [])]
        for u in si.on_update:
            lane = dma_lane.setdefault(u.ant_name, [])
            prev = lane[-1][0] if lane else 0
            lane.append((prev + u.update_value, waits))

    def implied_by_proc(sem_name, value, index, proc_target):
        cums, per_sem = index
        if sem_name not in per_sem:
            return False
        j = bisect.bisect_left(cums, proc_target)
        if j == len(cums):
            j -= 1
        poss, pmax = per_sem[sem_name]
        k = bisect.bisect_right(poss, j) - 1
        return k >= 0 and pmax[k] >= value

    def implied_by(w, other, depth=0):
        s, v = w
        os_, ov = other
        if os_ == s and ov >= v:
            return True
        if os_ in sem_to_index:
            return implied_by_proc(s, v, sem_to_index[os_], ov)
        if os_ in dma_lane:
            for cum, waits in dma_lane[os_]:
                for ww in waits:
                    if ww[0] == s and ww[1] >= v:
                        return True
                    if depth < 2 and implied_by(w, ww, depth + 1):
                        return True
                if cum >= ov:
                    break
        return False

    # Pass 2: prune, walking the block in program order.
    own_cum = {}
    satisfied = {}
    for inst in blk.instructions:
        si = inst.sync_info
        eng = inst.engine
        is_compute = eng in compute_engines and inst.opcode not in (
            "Drain", "NoOp", "EventSemaphore")
        waits = [(w.ant_name, w.wait_value) for w in (si.on_wait or [])] \
            if si else []
        # ISA sync-slot budget per instruction kind (waits beyond this
        # fail walrus codegen): DMAs get 1, ACT ops 2, DVE compute 3.
        if inst.opcode == "DMACopy":
            limit = 1
        elif inst.opcode == "Activation":
            limit = 2
        else:
            limit = 3
        if waits and si:
            own_sem = eng_sems.get(eng)
            cum = own_cum.get(eng, 0)
            sat = satisfied.setdefault(eng, {})
            kept_idx = list(range(len(waits)))
            if is_compute:
                # (b) duplicate of an earlier same-engine wait: always safe,
                # no ordering edge is lost (engine issue order is program
                # order and the earlier instruction already blocked on it).
                for i in list(kept_idx):
                    s, v = waits[i]
                    if sat.get(s, -1) >= v:
                        kept_idx.remove(i)
            if inst.opcode in ("DMACopy", "Drain") or \
                    (is_compute and len(kept_idx) > 1):
                # (c) transitively implied waits: the ordering chain still
                # exists through the retained wait, so vector clocks (and
                # the race detector) are preserved.
                changed = True
                while changed:
                    changed = False
                    for i in list(kept_idx):
                        others = [waits[j] for j in kept_idx if j != i]
                        if any(implied_by(waits[i], o) for o in others):
                            kept_idx.remove(i)
                            changed = True
                            break
            if is_compute and len(kept_idx) > limit:
                # (a) last resort while over the slot budget: drop own-sem
                # waits already passed by the engine's in-order execution.
                for i in sorted(
                        kept_idx,
                        key=lambda i: cum - waits[i][1],
                        reverse=True):
                    if len(kept_idx) <= limit:
                        break
                    s, v = waits[i]
                    if s == own_sem and v <= cum - 1:
                        kept_idx.remove(i)
            consolidated = False
            if inst.opcode == "DMACopy" and len(kept_idx) > 1:
                # Walrus allows a single wait on DMAs. Anchor on one
                # proc-sem wait and raise its value just enough that the
                # anchor engine's in-order stream implies the others.
                for i in kept_idx:
                    s_i, v_i = waits[i]
                    if s_i not in sem_to_index:
                        continue
                    cums, per_sem = sem_to_index[s_i]
                    need = v_i
                    ok = True
                    for j in kept_idx:
                        if j == i:
                            continue
                        s_j, v_j = waits[j]
                        poss, pmax = per_sem.get(s_j, ([], []))
                        k = bisect.bisect_left(pmax, v_j)
                        if k == len(poss):
                            ok = False
                            break
                        need = max(need, cums[poss[k]])
                    if ok and need - v_i <= 8:
                        wobj = si.on_wait[i]
                        wobj.wait_value = need
                        si.on_wait = [wobj]
                        consolidated = True
                        break
            if not consolidated and len(kept_idx) < len(waits):
                si.on_wait = [si.on_wait[i] for i in kept_idx]
            if is_compute:
                for s, v in waits:
                    sat[s] = max(sat.get(s, -1), v)
        if si and si.on_update and eng in compute_engines:
            own_sem = eng_sems.get(eng)
            for u in si.on_update:
                if u.ant_name == own_sem:
                    own_cum[eng] = own_cum.get(eng, 0) + u.update_value


def _get_program():
    key = (T, G)
    if key not in _PROGRAM_CACHE:
        _PROGRAM_CACHE[key] = build_program()
    return _PROGRAM_CACHE[key]


def fold_weights(W_lin0, W_lin1, W_lin2, W_tp0, W_tp1, W_tp2):
    """Collapse both weight stages + norms into the device layout awt[2304]:
    [w0n | w1n | w1n | w2n x5 | w1n] matching the kernel's section order
    (3 DVE-direct sections, then the 6 ACT sections l2_i0..4, l1_i2)."""
    lin_norm = 1.0 / np.sqrt(np.float64(MUL))
    alpha = 1.0 / np.sqrt(3.0 * MUL)
    w0 = W_lin0.astype(np.float64) @ W_tp0.astype(np.float64)
    w1 = W_lin1.astype(np.float64) @ W_tp1.astype(np.float64)
    w2 = W_lin2.astype(np.float64) @ W_tp2.astype(np.float64)
    w0n = w0 * (alpha * lin_norm)
    w1n = w1 * (alpha * lin_norm / SQ3)
    w2n = w2 * (alpha * lin_norm / SQ5)
    awt = np.concatenate([w0n, w1n, w1n, np.tile(w2n, 5), w1n])
    assert awt.shape == (K,)
    return awt.astype(np.float32)


def kernel(**inputs):
    global LAST_RESULTS
    from concourse.bass_utils import run_bass_kernel_spmd

    node_feats = np.asarray(inputs["node_feats"], dtype=np.float32)
    charges = np.asarray(inputs["charges"], dtype=np.float32)
    A = fold_weights(
        np.asarray(inputs["W_lin0"], dtype=np.float32),
        np.asarray(inputs["W_lin1"], dtype=np.float32),
        np.asarray(inputs["W_lin2"], dtype=np.float32),
        np.asarray(inputs["W_tp0"], dtype=np.float32),
        np.asarray(inputs["W_tp1"], dtype=np.float32),
        np.asarray(inputs["W_tp2"], dtype=np.float32),
    )
    arep = np.ascontiguousarray(np.broadcast_to(A, (P, K)))

    node_feats = np.ascontiguousarray(node_feats)
    charges = np.ascontiguousarray(charges)

    in_maps = []
    for c in range(N_CORES):
        lo, hi = c * N_SHARD, (c + 1) * N_SHARD
        in_maps.append({
            "x": node_feats[lo:hi].reshape(P, T * K),
            "arep": arep,
            "ch": charges[lo:hi].reshape(P, T * 9),
        })

    nc = _get_program()
    res = run_bass_kernel_spmd(nc, in_maps, list(range(N_CORES)))
    LAST_RESULTS = res
    out = np.concatenate(
        [np.asarray(res.results[c]["en"]).reshape(N_SHARD) for c in range(N_CORES)]
    )
    return out


# revision 15
# speedup vs baseline: 1.0750x; 1.0750x over previous
"""Trainium2 Bass kernel for nn_ChargesReadoutBlock.

Math: the reference is
    y_l = (x_l @ W_lin_l) / sqrt(256)            (per irrep l = 0e, 1o, 2e)
    p_l = einsum('zui,u,zi->z', y_l, W_tp_l, c_l) / sqrt(2l+1)
    energy = (p_0 + p_1 + p_2) / sqrt(3*256)

Everything is linear, so the two weight stages collapse on the host:
    w_l = W_lin_l @ W_tp_l                       ([256] each)
    energy[z] = sum_k node_feats[z, k] * A[k] * C[z, j(k)]
where A folds w_l and all scalar norms (per-k weight, shared across nodes)
and C[z, j] are the 9 per-node charge components (j = irrep slot of k).

Device kernel (data-parallel over nodes, 8 cores x 8192 nodes): nodes live
on partitions (128) x 64 node-tiles. The 9 irrep-component sections are
split across two engines so each stays under the HBM DMA roofline:

  ACT (6 sections: all 5 of l2, plus l1 i=2): per section
      y[:, s*256:(s+1)*256] = x_section * C[z, j]     (activation, scale=AP)
  DVE (1 batched reduce + 3 direct sections):
      big STT:  accum  = sum(y * awt_grouped)          FD=1536, one op/tile
      3x STT:   accum_j = sum((x_sec * C[z,j]) * w_sec) FD=256  (l0, l1 i=0,1)
      per chunk: tensor_reduce over the 4 accum slots -> energy

awt layout (host-folded): [w0n | w1n | w1n | w2n x5 | w1n], so the big STT's
in1 is contiguous and matches y's section order [l2_i0..4, l1_i2].
All heavy traffic is the single streaming read of node_feats
(contiguous 73.7KB/partition DMA chunks).
"""

import numpy as np

N_NODES = 65536
N_CORES = 8
MUL = 256
K = 9 * MUL            # 2304 features per node
P = 128                # SBUF partitions
N_SHARD = N_NODES // N_CORES   # 8192 nodes per core
T = N_SHARD // P       # 64 node-tiles per partition (node = p*T + t)
G = 4                  # node-tiles per DMA chunk
SQ3 = float(np.sqrt(3.0))
SQ5 = float(np.sqrt(5.0))

_PROGRAM_CACHE = {}
LAST_RESULTS = None    # BassKernelResults of the most recent kernel() call


def build_program(t_tiles=T, g_chunk=G, repeat=1, hw_loop=1,
                  skip_compute=False, dma_once=False):
    """Build the SPMD Bass program (same program for every core).

    repeat > 1 unrolls the whole body `repeat` times (straight-line, same
    buffers, identical output) — used only by the timing harness to
    amortize host dispatch overhead out of the measurement. hw_loop > 1
    additionally wraps the unrolled body in a device-side For_i loop
    (cheap to compile; pays one all-engine barrier per back edge, which
    `repeat` amortizes).
    """
    import concourse.bass as bass
    import concourse.tile as tile
    from concourse import mybir

    f32 = mybir.dt.float32
    mult = mybir.AluOpType.mult
    copy_fn = mybir.ActivationFunctionType.Copy

    nc = bass.Bass(trn_type="TRN2", debug=False, name="charges_readout")
    x = nc.dram_tensor("x", [P, t_tiles * K], f32, kind="ExternalInput").ap()
    arep = nc.dram_tensor("arep", [P, K], f32, kind="ExternalInput").ap()
    ch = nc.dram_tensor("ch", [P, t_tiles * 9], f32, kind="ExternalInput").ap()
    en = nc.dram_tensor("en", [P, t_tiles], f32, kind="ExternalOutput").ap()

    n_chunks = t_tiles // g_chunk

    with tile.TileContext(nc) as tc:
        with tc.tile_pool(name="const", bufs=1) as cpool, \
             tc.tile_pool(name="xp", bufs=2) as xpool, \
             tc.tile_pool(name="yp", bufs=2) as ypool, \
             tc.tile_pool(name="scrp", bufs=2) as spool, \
             tc.tile_pool(name="s4p", bufs=2) as s4pool:

            awt_t = cpool.tile([P, K], f32)
            nc.sync.dma_start(out=awt_t[:], in_=arep[:, :])
            ch_t = cpool.tile([P, t_tiles * 9], f32)
            nc.sync.dma_start(out=ch_t[:], in_=ch[:, :])
            en_t = cpool.tile([P, t_tiles], f32)

            # Wait-collectors: absorb the const-DMA completion waits on cheap
            # copy ops so the first real compute op on each engine doesn't
            # accumulate more sync-wait slots than its ISA struct allows.
            dummy = cpool.tile([P, 3], f32)
            nc.vector.tensor_copy(dummy[:, 0:1], awt_t[:, 0:1])
            nc.vector.tensor_copy(dummy[:, 1:2], ch_t[:, 0:1])
            nc.scalar.copy(dummy[:, 2:3], ch_t[:, 1:2])
            if skip_compute:
                nc.vector.memset(en_t[:], 0.0)

            xg_cache = []

            def emit_rep():
                for c in range(n_chunks):
                    # dma_once: load only the first two chunks, then cycle
                    # those buffers for all compute (compute-only timing probe)
                    if dma_once and len(xg_cache) >= 2:
                        xg = xg_cache[c % 2]
                    else:
                        xg = xpool.tile([P, g_chunk * K], f32)
                        nc.sync.dma_start(
                            out=xg[:],
                            in_=x[:, c * g_chunk * K:(c + 1) * g_chunk * K]
                        )
                        if dma_once:
                            xg_cache.append(xg)
                    if skip_compute:
                        # tiny DVE read per chunk keeps the reader-release
                        # wait chain intact (prunable single-wait DMAs)
                        sink = spool.tile([P, 1], f32)
                        nc.vector.tensor_copy(sink[:], xg[:, 0:1])
                        continue
                    s4c = s4pool.tile([P, g_chunk * 5], f32)
                    yb = ypool.tile([P, g_chunk * 5 * MUL], f32)
                    ylast = g_chunk * 5 * MUL
                    # ACT wait-collectors (ISA allows 1 wait per ACT op):
                    # first absorbs the ybig WAR release (vs the big STTs of
                    # the chunk that last used this buffer) at its maximum
                    # value; second absorbs this chunk's x-DMA completion.
                    # Both write junk into the last tile's last section,
                    # which the real ops fully overwrite before it is read.
                    nc.scalar.activation(
                        out=yb[:, ylast - 1:ylast], in_=ch_t[:, 0:1],
                        func=copy_fn)
                    nc.scalar.activation(
                        out=yb[:, ylast - 2:ylast - 1], in_=xg[:, 0:1],
                        func=copy_fn)
                    for tsub in range(g_chunk):
                        t_idx = c * g_chunk + tsub
                        t9 = t_idx * 9
                        base = tsub * K
                        y = yb[:, tsub * 5 * MUL:(tsub + 1) * 5 * MUL]
                        scr = spool.tile([P, K], f32)
                        x1 = xg[:, base + MUL:base + 4 * MUL].rearrange(
                            "p (u i) -> p i u", i=3)
                        x2 = xg[:, base + 4 * MUL:base + 9 * MUL].rearrange(
                            "p (u i) -> p i u", i=5)
                        # ACT: charge-scale the 5 l2 sections into y
                        for s in range(5):
                            nc.scalar.activation(
                                out=y[:, s * MUL:(s + 1) * MUL],
                                in_=x2[:, s, :],
                                func=copy_fn,
                                scale=ch_t[:, t9 + 4 + s:t9 + 5 + s],
                            )
                        # DVE: 4 direct fused sections (l0, l1 i=0,1,2)
                        direct = [
                            (xg[:, base:base + MUL], 0, 0),
                            (x1[:, 0, :], 1, 1),
                            (x1[:, 1, :], 2, 2),
                            (x1[:, 2, :], 3, 3),
                        ]
                        for xi, ci, j in direct:
                            nc.vector.scalar_tensor_tensor(
                                out=scr[:, j * MUL:(j + 1) * MUL],
                                in0=xi,
                                scalar=ch_t[:, t9 + ci:t9 + ci + 1],
                                in1=awt_t[:, j * MUL:(j + 1) * MUL],
                                op0=mult,
                                op1=mult,
                                accum_out=s4c[:, tsub * 5 + 1 + j:tsub * 5 + 2 + j],
                            )
                        # DVE: one batched reduce over ACT's 5 sections
                        nc.vector.scalar_tensor_tensor(
                            out=scr[:, 4 * MUL:9 * MUL],
                            in0=y[:],
                            scalar=1.0,
                            in1=awt_t[:, 4 * MUL:9 * MUL],
                            op0=mult,
                            op1=mult,
                            accum_out=s4c[:, tsub * 5:tsub * 5 + 1],
                        )
                    # one batched reduce per chunk: [P, g, 5] -> [P, g]
                    nc.vector.tensor_reduce(
                        out=en_t[:, c * g_chunk:(c + 1) * g_chunk],
                        in_=s4c[:].rearrange("p (g j) -> p g j", j=5),
                        axis=mybir.AxisListType.X,
                        op=mybir.AluOpType.add,
                    )

            if hw_loop > 1:
                with tc.For_i(0, hw_loop, 1):
                    for _rep in range(repeat):
                        emit_rep()
            else:
                for _rep in range(repeat):
                    emit_rep()
            nc.sync.dma_start(out=en[:, :], in_=en_t[:])
    _prune_implied_dma_waits(nc)
    return nc


def _prune_implied_dma_waits(nc):
    """Reduce sync waits to what the ISA wait slots allow.

    Walrus in this toolchain rejects DMAs with more than one sync wait and
    DVE/ACT compute ops with more than two. Tile's wait emission is per-proc
    minimal but not transitively minimal (documented), so we drop waits we
    can prove implied:
      (a) own-proc-sem waits with value <= updates already completed by the
          same in-order engine (same-engine WAW/WAR ordering);
      (b) waits an earlier same-engine instruction already carried (issue
          order is program order, waits are satisfied before issue);
      (c) waits implied transitively through another carried wait (the
          instruction that pushed sem X to the waited value itself carried,
          or transitively implies, the other wait);
      (d) two-reader DMA releases: anchor on one proc-sem wait, raising it
          minimally so that engine's in-order stream implies the other.
    Each basic block is processed as an independent region: For_i loop
    bodies repeat with per-iteration semaphore resets, so cross-block state
    must not leak. Tile's own loop machinery (NoOp/Drain barriers in
    reset/exit blocks) is left untouched.
    """
    import bisect

    from concourse import mybir

    # The program is straight-line (no For_i): treat all blocks as one
    # linear region. (Looped programs would need per-iteration state
    # resets; the For_i path is unused — walrus rejects its barrier ops.)
    class _Lin:
        instructions = [i for b in nc.m.functions[0].blocks
                        for i in b.instructions]

    _prune_block(_Lin, mybir, bisect)
    return nc


def _prune_block(blk, mybir, bisect):
    compute_engines = (mybir.EngineType.DVE, mybir.EngineType.Activation)

    # Pass 1: per-engine instruction streams in program order with
    # cumulative proc-sem ticks and the waits each instruction carried.
    eng_sems = {}
    eng_streams = {}
    eng_cum = {}
    for inst in blk.instructions:
        eng = inst.engine
        if eng not in compute_engines:
            continue
        si = inst.sync_info
        waits = [(w.ant_name, w.wait_value) for w in (si.on_wait or [])] \
            if si else []
        if si and si.on_update:
            for u in si.on_update:
                if u.ant_name.startswith(("DVE", "ACT", "Activation")):
                    eng_sems[eng] = u.ant_name
                    eng_cum[eng] = eng_cum.get(eng, 0) + u.update_value
        eng_streams.setdefault(eng, []).append((eng_cum.get(eng, 0), waits))

    # Index each engine stream for O(log n) implication queries.
    sem_to_index = {}
    for eng, sem in eng_sems.items():
        stream = eng_streams[eng]
        cums = [cum for cum, _ in stream]
        per_sem = {}
        for idx, (_, waits) in enumerate(stream):
            for s, v in waits:
                poss, pmax = per_sem.setdefault(s, ([], []))
                poss.append(idx)
                pmax.append(max(v, pmax[-1]) if pmax else v)
        sem_to_index[sem] = (cums, per_sem)

    # Per DMA-lane sem: (cumulative completion value, carried waits).
    dma_lane = {}
    for inst in blk.instructions:
        if inst.opcode != "DMACopy":
            continue
        si = inst.sync_info
        if not si or not si.on_update:
            continue
        waits = [(w.ant_name, w.wait_value) for w in (si.on_wait or [])]
        for u in si.on_update:
            lane = dma_lane.setdefault(u.ant_name, [])
            prev = lane[-1][0] if lane else 0
            lane.append((prev + u.update_value, waits))

    def implied_by_proc(sem_name, value, index, proc_target):
        cums, per_sem = index
        if sem_name not in per_sem:
            return False
        j = bisect.bisect_left(cums, proc_target)
        if j == len(cums):
            j -= 1
        poss, pmax = per_sem[sem_name]
        k = bisect.bisect_right(poss, j) - 1
        return k >= 0 and pmax[k] >= value

    def implied_by(w, other, depth=0):
        s, v = w
        os_, ov = other
        if os_ == s and ov >= v:
            return True
        if os_ in sem_to_index:
            return implied_by_proc(s, v, sem_to_index[os_], ov)
        if os_ in dma_lane:
            for cum, waits in dma_lane[os_]:
                for ww in waits:
                    if ww[0] == s and ww[1] >= v:
                        return True
                    if depth < 2 and implied_by(w, ww, depth + 1):
                        return True
                if cum >= ov:
                    break
        return False

    # Pass 2: prune, walking the block in program order.
    own_cum = {}
    satisfied = {}
    for inst in blk.instructions:
        si = inst.sync_info
        eng = inst.engine
        is_compute = eng in compute_engines and inst.opcode not in (
            "Drain", "NoOp", "EventSemaphore")
        waits = [(w.ant_name, w.wait_value) for w in (si.on_wait or [])] \
            if si else []
        # ISA sync-slot budget per instruction kind (waits beyond this
        # fail walrus codegen): DMAs get 1, ACT ops 2, DVE compute 3.
        if inst.opcode == "DMACopy":
            limit = 1
        elif inst.opcode == "Activation":
            limit = 2
        else:
            limit = 3
        if waits and si:
            own_sem = eng_sems.get(eng)
            cum = own_cum.get(eng, 0)
            sat = satisfied.setdefault(eng, {})
            kept_idx = list(range(len(waits)))
            if is_compute:
                # (b) duplicate of an earlier same-engine wait: always safe,
                # no ordering edge is lost (engine issue order is program
                # order and the earlier instruction already blocked on it).
                for i in list(kept_idx):
                    s, v = waits[i]
                    if sat.get(s, -1) >= v:
                        kept_idx.remove(i)
            if inst.opcode in ("DMACopy", "Drain") or \
                    (is_compute and len(kept_idx) > 1):
                # (c) transitively implied waits: the ordering chain still
                # exists through the retained wait, so vector clocks (and
                # the race detector) are preserved.
                changed = True
                while changed:
                    changed = False
                    for i in list(kept_idx):
                        others = [waits[j] for j in kept_idx if j != i]
                        if any(implied_by(waits[i], o) for o in others):
                            kept_idx.remove(i)
                            changed = True
                            break
            if is_compute and len(kept_idx) > limit:
                # (a) last resort while over the slot budget: drop own-sem
                # waits already passed by the engine's in-order execution.
                for i in sorted(
                        kept_idx,
                        key=lambda i: cum - waits[i][1],
                        reverse=True):
                    if len(kept_idx) <= limit:
                        break
                    s, v = waits[i]
                    if s == own_sem and v <= cum - 1:
                        kept_idx.remove(i)
            consolidated = False
            if inst.opcode == "DMACopy" and len(kept_idx) > 1:
                # Walrus allows a single wait on DMAs. Anchor on one
                # proc-sem wait and raise its value just enough that the
                # anchor engine's in-order stream implies the others.
                for i in kept_idx:
                    s_i, v_i = waits[i]
                    if s_i not in sem_to_index:
                        continue
                    cums, per_sem = sem_to_index[s_i]
                    need = v_i
                    ok = True
                    for j in kept_idx:
                        if j == i:
                            continue
                        s_j, v_j = waits[j]
                        poss, pmax = per_sem.get(s_j, ([], []))
                        k = bisect.bisect_left(pmax, v_j)
                        if k == len(poss):
                            ok = False
                            break
                        need = max(need, cums[poss[k]])
                    if ok and need - v_i <= 8:
                        wobj = si.on_wait[i]
                        wobj.wait_value = need
                        si.on_wait = [wobj]
                        consolidated = True
                        break
            if not consolidated and len(kept_idx) < len(waits):
                si.on_wait = [si.on_wait[i] for i in kept_idx]
            if is_compute:
                for s, v in waits:
                    sat[s] = max(sat.get(s, -1), v)
        if si and si.on_update and eng in compute_engines:
            own_sem = eng_sems.get(eng)
            for u in si.on_update:
                if u.ant_name == own_sem:
                    own_cum[eng] = own_cum.get(eng, 0) + u.update_value


def _get_program():
    key = (T, G)
    if key not in _PROGRAM_CACHE:
        _PROGRAM_CACHE[key] = build_program()
    return _PROGRAM_CACHE[key]


def fold_weights(W_lin0, W_lin1, W_lin2, W_tp0, W_tp1, W_tp2):
    """Collapse both weight stages + norms into the device layout awt[2304]:
    [w0n | w1n | w1n | w2n x5 | w1n] matching the kernel's section order
    (3 DVE-direct sections, then the 6 ACT sections l2_i0..4, l1_i2)."""
    lin_norm = 1.0 / np.sqrt(np.float64(MUL))
    alpha = 1.0 / np.sqrt(3.0 * MUL)
    w0 = W_lin0.astype(np.float64) @ W_tp0.astype(np.float64)
    w1 = W_lin1.astype(np.float64) @ W_tp1.astype(np.float64)
    w2 = W_lin2.astype(np.float64) @ W_tp2.astype(np.float64)
    w0n = w0 * (alpha * lin_norm)
    w1n = w1 * (alpha * lin_norm / SQ3)
    w2n = w2 * (alpha * lin_norm / SQ5)
    awt = np.concatenate([w0n, w1n, w1n, w1n, np.tile(w2n, 5)])
    assert awt.shape == (K,)
    return awt.astype(np.float32)


def kernel(**inputs):
    global LAST_RESULTS
    from concourse.bass_utils import run_bass_kernel_spmd

    node_feats = np.asarray(inputs["node_feats"], dtype=np.float32)
    charges = np.asarray(inputs["charges"], dtype=np.float32)
    A = fold_weights(
        np.asarray(inputs["W_lin0"], dtype=np.float32),
        np.asarray(inputs["W_lin1"], dtype=np.float32),
        np.asarray(inputs["W_lin2"], dtype=np.float32),
        np.asarray(inputs["W_tp0"], dtype=np.float32),
        np.asarray(inputs["W_tp1"], dtype=np.float32),
        np.asarray(inputs["W_tp2"], dtype=np.float32),
    )
    arep = np.ascontiguousarray(np.broadcast_to(A, (P, K)))

    node_feats = np.ascontiguousarray(node_feats)
    charges = np.ascontiguousarray(charges)

    in_maps = []
    for c in range(N_CORES):
        lo, hi = c * N_SHARD, (c + 1) * N_SHARD
        in_maps.append({
            "x": node_feats[lo:hi].reshape(P, T * K),
            "arep": arep,
            "ch": charges[lo:hi].reshape(P, T * 9),
        })

    nc = _get_program()
    res = run_bass_kernel_spmd(nc, in_maps, list(range(N_CORES)))
    LAST_RESULTS = res
    out = np.concatenate(
        [np.asarray(res.results[c]["en"]).reshape(N_SHARD) for c in range(N_CORES)]
    )
    return out


# revision 16
# speedup vs baseline: 1.0823x; 1.0069x over previous
"""Trainium2 Bass kernel for nn_ChargesReadoutBlock.

Math: the reference is
    y_l = (x_l @ W_lin_l) / sqrt(256)            (per irrep l = 0e, 1o, 2e)
    p_l = einsum('zui,u,zi->z', y_l, W_tp_l, c_l) / sqrt(2l+1)
    energy = (p_0 + p_1 + p_2) / sqrt(3*256)

Everything is linear, so the two weight stages collapse on the host:
    w_l = W_lin_l @ W_tp_l                       ([256] each)
    energy[z] = sum_k node_feats[z, k] * A[k] * C[z, j(k)]
where A folds w_l and all scalar norms (per-k weight, shared across nodes)
and C[z, j] are the 9 per-node charge components (j = irrep slot of k).

Device kernel (data-parallel over nodes, 8 cores x 8192 nodes): nodes live
on partitions (128) x 64 node-tiles. The 9 irrep-component sections are
split across two engines so each stays under the HBM DMA roofline:

  ACT (5 sections: all of l2): per section
      y[:, s*256:(s+1)*256] = x_section * C[z, j]     (activation, scale=AP)
  DVE (1 batched reduce + 4 direct sections):
      big STT:  accum  = sum(y * awt_grouped)          FD=1280, one op/tile
      4x STT:   accum_j = sum((x_sec * C[z,j]) * w_sec) FD=256 (l0, l1 i=0..2)
      per chunk: tensor_reduce over the 5 accum slots -> energy

awt layout (host-folded): [w0n | w1n | w1n | w1n | w2n x5], so the big STT's
in1 is contiguous and matches y's section order [l2_i0..4]. ACT ops are
capped at one ISA sync-wait slot; two per-chunk wait-collector dummies plus
the wait pruner keep every real ACT op at zero waits. All heavy traffic is
the single streaming read of node_feats (36.9KB/partition DMA chunks).
"""

import numpy as np

N_NODES = 65536
N_CORES = 8
MUL = 256
K = 9 * MUL            # 2304 features per node
P = 128                # SBUF partitions
N_SHARD = N_NODES // N_CORES   # 8192 nodes per core
T = N_SHARD // P       # 64 node-tiles per partition (node = p*T + t)
G = 4                  # node-tiles per DMA chunk
SQ3 = float(np.sqrt(3.0))
SQ5 = float(np.sqrt(5.0))

_PROGRAM_CACHE = {}
LAST_RESULTS = None    # BassKernelResults of the most recent kernel() call


def build_program(t_tiles=T, g_chunk=G, repeat=1, hw_loop=1,
                  skip_compute=False, dma_once=False):
    """Build the SPMD Bass program (same program for every core).

    repeat > 1 unrolls the whole body `repeat` times (straight-line, same
    buffers, identical output) — used only by the timing harness to
    amortize host dispatch overhead out of the measurement. hw_loop > 1
    additionally wraps the unrolled body in a device-side For_i loop
    (cheap to compile; pays one all-engine barrier per back edge, which
    `repeat` amortizes).
    """
    import concourse.bass as bass
    import concourse.tile as tile
    from concourse import mybir

    f32 = mybir.dt.float32
    mult = mybir.AluOpType.mult
    copy_fn = mybir.ActivationFunctionType.Copy

    nc = bass.Bass(trn_type="TRN2", debug=False, name="charges_readout")
    x = nc.dram_tensor("x", [P, t_tiles * K], f32, kind="ExternalInput").ap()
    arep = nc.dram_tensor("arep", [P, K], f32, kind="ExternalInput").ap()
    ch = nc.dram_tensor("ch", [P, t_tiles * 9], f32, kind="ExternalInput").ap()
    en = nc.dram_tensor("en", [P, t_tiles], f32, kind="ExternalOutput").ap()

    n_chunks = t_tiles // g_chunk

    with tile.TileContext(nc) as tc:
        with tc.tile_pool(name="const", bufs=1) as cpool, \
             tc.tile_pool(name="xp", bufs=2) as xpool, \
             tc.tile_pool(name="yp", bufs=2) as ypool, \
             tc.tile_pool(name="scrp", bufs=2) as spool, \
             tc.tile_pool(name="s4p", bufs=2) as s4pool:

            awt_t = cpool.tile([P, K], f32)
            nc.sync.dma_start(out=awt_t[:], in_=arep[:, :])
            ch_t = cpool.tile([P, t_tiles * 9], f32)
            nc.sync.dma_start(out=ch_t[:], in_=ch[:, :])
            en_t = cpool.tile([P, t_tiles], f32)

            # Wait-collectors: absorb the const-DMA completion waits on cheap
            # copy ops so the first real compute op on each engine doesn't
            # accumulate more sync-wait slots than its ISA struct allows.
            dummy = cpool.tile([P, 3], f32)
            nc.vector.tensor_copy(dummy[:, 0:1], awt_t[:, 0:1])
            nc.vector.tensor_copy(dummy[:, 1:2], ch_t[:, 0:1])
            nc.scalar.copy(dummy[:, 2:3], ch_t[:, 1:2])
            if skip_compute:
                nc.vector.memset(en_t[:], 0.0)

            xg_cache = []

            def emit_rep():
                for c in range(n_chunks):
                    # dma_once: load only the first two chunks, then cycle
                    # those buffers for all compute (compute-only timing probe)
                    if dma_once and len(xg_cache) >= 2:
                        xg = xg_cache[c % 2]
                    else:
                        xg = xpool.tile([P, g_chunk * K], f32)
                        nc.sync.dma_start(
                            out=xg[:],
                            in_=x[:, c * g_chunk * K:(c + 1) * g_chunk * K]
                        )
                        if dma_once:
                            xg_cache.append(xg)
                    if skip_compute:
                        # tiny DVE read per chunk keeps the reader-release
                        # wait chain intact (prunable single-wait DMAs)
                        sink = spool.tile([P, 1], f32)
                        nc.vector.tensor_copy(sink[:], xg[:, 0:1])
                        continue
                    s4c = s4pool.tile([P, g_chunk * 5], f32)
                    yb = ypool.tile([P, g_chunk * 5 * MUL], f32)
                    ylast = g_chunk * 5 * MUL
                    # ACT wait-collectors (ISA allows 1 wait per ACT op):
                    # first absorbs the ybig WAR release (vs the big STTs of
                    # the chunk that last used this buffer) at its maximum
                    # value; second absorbs this chunk's x-DMA completion.
                    # Both write junk into the last tile's last section,
                    # which the real ops fully overwrite before it is read.
                    nc.scalar.activation(
                        out=yb[:, ylast - 1:ylast], in_=ch_t[:, 0:1],
                        func=copy_fn)
                    nc.scalar.activation(
                        out=yb[:, ylast - 2:ylast - 1], in_=xg[:, 0:1],
                        func=copy_fn)
                    for tsub in range(g_chunk):
                        t_idx = c * g_chunk + tsub
                        t9 = t_idx * 9
                        base = tsub * K
                        y = yb[:, tsub * 5 * MUL:(tsub + 1) * 5 * MUL]
                        scr = spool.tile([P, K], f32)
                        x1 = xg[:, base + MUL:base + 4 * MUL].rearrange(
                            "p (u i) -> p i u", i=3)
                        x2 = xg[:, base + 4 * MUL:base + 9 * MUL].rearrange(
                            "p (u i) -> p i u", i=5)
                        # ACT: charge-scale the 5 l2 sections into y
                        for s in range(5):
                            nc.scalar.activation(
                                out=y[:, s * MUL:(s + 1) * MUL],
                                in_=x2[:, s, :],
                                func=copy_fn,
                                scale=ch_t[:, t9 + 4 + s:t9 + 5 + s],
                            )
                        # DVE: 4 direct fused sections (l0, l1 i=0,1,2)
                        direct = [
                            (xg[:, base:base + MUL], 0, 0),
                            (x1[:, 0, :], 1, 1),
                            (x1[:, 1, :], 2, 2),
                            (x1[:, 2, :], 3, 3),
                        ]
                        for xi, ci, j in direct:
                            nc.vector.scalar_tensor_tensor(
                                out=scr[:, j * MUL:(j + 1) * MUL],
                                in0=xi,
                                scalar=ch_t[:, t9 + ci:t9 + ci + 1],
                                in1=awt_t[:, j * MUL:(j + 1) * MUL],
                                op0=mult,
                                op1=mult,
                                accum_out=s4c[:, tsub * 5 + 1 + j:tsub * 5 + 2 + j],
                            )
                        # DVE: one batched reduce over ACT's 5 sections
                        nc.vector.scalar_tensor_tensor(
                            out=scr[:, 4 * MUL:9 * MUL],
                            in0=y[:],
                            scalar=1.0,
                            in1=awt_t[:, 4 * MUL:9 * MUL],
                            op0=mult,
                            op1=mult,
                            accum_out=s4c[:, tsub * 5:tsub * 5 + 1],
                        )
                    # one batched reduce per chunk: [P, g, 5] -> [P, g]
                    nc.vector.tensor_reduce(
                        out=en_t[:, c * g_chunk:(c + 1) * g_chunk],
                        in_=s4c[:].rearrange("p (g j) -> p g j", j=5),
                        axis=mybir.AxisListType.X,
                        op=mybir.AluOpType.add,
                    )

            if hw_loop > 1:
                with tc.For_i(0, hw_loop, 1):
                    for _rep in range(repeat):
                        emit_rep()
            else:
                for _rep in range(repeat):
                    emit_rep()
            nc.sync.dma_start(out=en[:, :], in_=en_t[:])
    _prune_implied_dma_waits(nc)
    return nc


def _prune_implied_dma_waits(nc):
    """Reduce sync waits to what the ISA wait slots allow.

    Walrus in this toolchain rejects DMAs with more than one sync wait and
    DVE/ACT compute ops with more than two. Tile's wait emission is per-proc
    minimal but not transitively minimal (documented), so we drop waits we
    can prove implied:
      (a) own-proc-sem waits with value <= updates already completed by the
          same in-order engine (same-engine WAW/WAR ordering);
      (b) waits an earlier same-engine instruction already carried (issue
          order is program order, waits are satisfied before issue);
      (c) waits implied transitively through another carried wait (the
          instruction that pushed sem X to the waited value itself carried,
          or transitively implies, the other wait);
      (d) two-reader DMA releases: anchor on one proc-sem wait, raising it
          minimally so that engine's in-order stream implies the other.
    Each basic block is processed as an independent region: For_i loop
    bodies repeat with per-iteration semaphore resets, so cross-block state
    must not leak. Tile's own loop machinery (NoOp/Drain barriers in
    reset/exit blocks) is left untouched.
    """
    import bisect

    from concourse import mybir

    # The program is straight-line (no For_i): treat all blocks as one
    # linear region. (Looped programs would need per-iteration state
    # resets; the For_i path is unused — walrus rejects its barrier ops.)
    class _Lin:
        instructions = [i for b in nc.m.functions[0].blocks
                        for i in b.instructions]

    _prune_block(_Lin, mybir, bisect)
    return nc


def _prune_block(blk, mybir, bisect):
    compute_engines = (mybir.EngineType.DVE, mybir.EngineType.Activation)

    # Pass 1: per-engine instruction streams in program order with
    # cumulative proc-sem ticks and the waits each instruction carried.
    eng_sems = {}
    eng_streams = {}
    eng_cum = {}
    for inst in blk.instructions:
        eng = inst.engine
        if eng not in compute_engines:
            continue
        si = inst.sync_info
        waits = [(w.ant_name, w.wait_value) for w in (si.on_wait or [])] \
            if si else []
        if si and si.on_update:
            for u in si.on_update:
                if u.ant_name.startswith(("DVE", "ACT", "Activation")):
                    eng_sems[eng] = u.ant_name
                    eng_cum[eng] = eng_cum.get(eng, 0) + u.update_value
        eng_streams.setdefault(eng, []).append((eng_cum.get(eng, 0), waits))

    # Index each engine stream for O(log n) implication queries.
    sem_to_index = {}
    for eng, sem in eng_sems.items():
        stream = eng_streams[eng]
        cums = [cum for cum, _ in stream]
        per_sem = {}
        for idx, (_, waits) in enumerate(stream):
            for s, v in waits:
                poss, pmax = per_sem.setdefault(s, ([], []))
                poss.append(idx)
                pmax.append(max(v, pmax[-1]) if pmax else v)
        sem_to_index[sem] = (cums, per_sem)

    # Per DMA-lane sem: (cumulative completion value, carried waits).
    dma_lane = {}
    for inst in blk.instructions:
        if inst.opcode != "DMACopy":
            continue
        si = inst.sync_info
        if not si or not si.on_update:
            continue
        waits = [(w.ant_name, w.wait_value) for w in (si.on_wait or [])]
        for u in si.on_update:
            lane = dma_lane.setdefault(u.ant_name, [])
            prev = lane[-1][0] if lane else 0
            lane.append((prev + u.update_value, waits))

    def implied_by_proc(sem_name, value, index, proc_target):
        cums, per_sem = index
        if sem_name not in per_sem:
            return False
        j = bisect.bisect_left(cums, proc_target)
        if j == len(cums):
            j -= 1
        poss, pmax = per_sem[sem_name]
        k = bisect.bisect_right(poss, j) - 1
        return k >= 0 and pmax[k] >= value

    def implied_by(w, other, depth=0):
        s, v = w
        os_, ov = other
        if os_ == s and ov >= v:
            return True
        if os_ in sem_to_index:
            return implied_by_proc(s, v, sem_to_index[os_], ov)
        if os_ in dma_lane:
            for cum, waits in dma_lane[os_]:
                for ww in waits:
                    if ww[0] == s and ww[1] >= v:
                        return True
                    if depth < 2 and implied_by(w, ww, depth + 1):
                        return True
                if cum >= ov:
                    break
        return False

    # Pass 2: prune, walking the block in program order.
    own_cum = {}
    satisfied = {}
    for inst in blk.instructions:
        si = inst.sync_info
        eng = inst.engine
        is_compute = eng in compute_engines and inst.opcode not in (
            "Drain", "NoOp", "EventSemaphore")
        waits = [(w.ant_name, w.wait_value) for w in (si.on_wait or [])] \
            if si else []
        # ISA sync-slot budget per instruction kind (waits beyond this
        # fail walrus codegen): DMAs get 1, ACT ops 2, DVE compute 3.
        if inst.opcode == "DMACopy":
            limit = 1
        elif inst.opcode == "Activation":
            limit = 2
        else:
            limit = 3
        if waits and si:
            own_sem = eng_sems.get(eng)
            cum = own_cum.get(eng, 0)
            sat = satisfied.setdefault(eng, {})
            kept_idx = list(range(len(waits)))
            if is_compute:
                # (b) duplicate of an earlier same-engine wait: always safe,
                # no ordering edge is lost (engine issue order is program
                # order and the earlier instruction already blocked on it).
                for i in list(kept_idx):
                    s, v = waits[i]
                    if sat.get(s, -1) >= v:
                        kept_idx.remove(i)
            if inst.opcode in ("DMACopy", "Drain") or \
                    (is_compute and len(kept_idx) > 1):
                # (c) transitively implied waits: the ordering chain still
                # exists through the retained wait, so vector clocks (and
                # the race detector) are preserved.
                changed = True
                while changed:
                    changed = False
                    for i in list(kept_idx):
                        others = [waits[j] for j in kept_idx if j != i]
                        if any(implied_by(waits[i], o) for o in others):
                            kept_idx.remove(i)
                            changed = True
                            break
            if is_compute and len(kept_idx) > limit:
                # (a) last resort while over the slot budget: drop own-sem
                # waits already passed by the engine's in-order execution.
                for i in sorted(
                        kept_idx,
                        key=lambda i: cum - waits[i][1],
                        reverse=True):
                    if len(kept_idx) <= limit:
                        break
                    s, v = waits[i]
                    if s == own_sem and v <= cum - 1:
                        kept_idx.remove(i)
            consolidated = False
            if inst.opcode == "DMACopy" and len(kept_idx) > 1:
                # Walrus allows a single wait on DMAs. Anchor on one
                # proc-sem wait and raise its value just enough that the
                # anchor engine's in-order stream implies the others.
                for i in kept_idx:
                    s_i, v_i = waits[i]
                    if s_i not in sem_to_index:
                        continue
                    cums, per_sem = sem_to_index[s_i]
                    need = v_i
                    ok = True
                    for j in kept_idx:
                        if j == i:
                            continue
                        s_j, v_j = waits[j]
                        poss, pmax = per_sem.get(s_j, ([], []))
                        k = bisect.bisect_left(pmax, v_j)
                        if k == len(poss):
                            ok = False
                            break
                        need = max(need, cums[poss[k]])
                    if ok and need - v_i <= 8:
                        wobj = si.on_wait[i]
                        wobj.wait_value = need
                        si.on_wait = [wobj]
                        consolidated = True
                        break
            if not consolidated and len(kept_idx) < len(waits):
                si.on_wait = [si.on_wait[i] for i in kept_idx]
            if is_compute:
                for s, v in waits:
                    sat[s] = max(sat.get(s, -1), v)
        if si and si.on_update and eng in compute_engines:
            own_sem = eng_sems.get(eng)
            for u in si.on_update:
                if u.ant_name == own_sem:
                    own_cum[eng] = own_cum.get(eng, 0) + u.update_value


def _get_program():
    key = (T, G)
    if key not in _PROGRAM_CACHE:
        _PROGRAM_CACHE[key] = build_program()
    return _PROGRAM_CACHE[key]


def fold_weights(W_lin0, W_lin1, W_lin2, W_tp0, W_tp1, W_tp2):
    """Collapse both weight stages + norms into the device layout awt[2304]:
    [w0n | w1n | w1n | w2n x5 | w1n] matching the kernel's section order
    (3 DVE-direct sections, then the 6 ACT sections l2_i0..4, l1_i2)."""
    lin_norm = 1.0 / np.sqrt(np.float64(MUL))
    alpha = 1.0 / np.sqrt(3.0 * MUL)
    w0 = W_lin0.astype(np.float64) @ W_tp0.astype(np.float64)
    w1 = W_lin1.astype(np.float64) @ W_tp1.astype(np.float64)
    w2 = W_lin2.astype(np.float64) @ W_tp2.astype(np.float64)
    w0n = w0 * (alpha * lin_norm)
    w1n = w1 * (alpha * lin_norm / SQ3)
    w2n = w2 * (alpha * lin_norm / SQ5)
    awt = np.concatenate([w0n, w1n, w1n, w1n, np.tile(w2n, 5)])
    assert awt.shape == (K,)
    return awt.astype(np.float32)


def kernel(**inputs):
    global LAST_RESULTS
    from concourse.bass_utils import run_bass_kernel_spmd

    node_feats = np.asarray(inputs["node_feats"], dtype=np.float32)
    charges = np.asarray(inputs["charges"], dtype=np.float32)
    A = fold_weights(
        np.asarray(inputs["W_lin0"], dtype=np.float32),
        np.asarray(inputs["W_lin1"], dtype=np.float32),
        np.asarray(inputs["W_lin2"], dtype=np.float32),
        np.asarray(inputs["W_tp0"], dtype=np.float32),
        np.asarray(inputs["W_tp1"], dtype=np.float32),
        np.asarray(inputs["W_tp2"], dtype=np.float32),
    )
    arep = np.ascontiguousarray(np.broadcast_to(A, (P, K)))

    node_feats = np.ascontiguousarray(node_feats)
    charges = np.ascontiguousarray(charges)

    in_maps = []
    for c in range(N_CORES):
        lo, hi = c * N_SHARD, (c + 1) * N_SHARD
        in_maps.append({
            "x": node_feats[lo:hi].reshape(P, T * K),
            "arep": arep,
            "ch": charges[lo:hi].reshape(P, T * 9),
        })

    nc = _get_program()
    res = run_bass_kernel_spmd(nc, in_maps, list(range(N_CORES)))
    LAST_RESULTS = res
    out = np.concatenate(
        [np.asarray(res.results[c]["en"]).reshape(N_SHARD) for c in range(N_CORES)]
    )
    return out
